# revision 117
# baseline (speedup 1.0000x reference)
"""CGConvBlock (3x CGConv + MLP/BatchNorm + graph LayerNorm) on 8 Trainium2 cores.

Sharding: nodes by graph (4 graphs/core, per-graph padded to GPAD rows);
edges by destination core (sorted by padded dst). Per layer:
  - dst-side preacts per 128-node block held as fp8 (hi, lo) pairs; ONE
    DoubleRow fp8 matmul (one-hot oh_dst duplicated over both k-tiles)
    broadcasts them to edges at half cycle cost
  - x[src] gathered bf16 via transposed dma_gather from the gather table
    (table rows are (j, block)-interleaved so a whole graph ships as one DMA)
  - gate/core pre-acts accumulate in PSUM [edge, 256]; the edge loop is
    software-pipelined: supergroup i's preacts issue before scatter(i-1) so
    the PE never waits on the ACT/DVE message chain
  - msg = softplus(core) * sigmoid(gate): ACT Exp/Ln + (1+u) on Pool (DVE on
    gather supergroups to dodge the descriptor-gen collision) + DVE recip/mult
  - scatter-add: matmul(lhsT=msg[e,c], rhs=oh_scat[e,n] fp8) -> [c, n] in PSUM
  - the BN h-pass (W1 matmul + sum/sumsq stats) is chunked and interleaved
    into the edge phase (one 512-wide chunk per supergroup, on PE+DVE)
  - node phase: BN scalars -> per-graph W2 matmuls + x3 + LN sums (pass A),
    LN scalars batched per graph pair, then y + transposes + one-DMA ship
  - AllGather of the bf16 node table for the next layer's gathers
"""
import sys

sys.path.insert(0, "/opt/trn_rl_repo")

import numpy as np
import ml_dtypes

N = 20000
E = 640000
C = 128
D = 64
H = 512
L = 3
G = 32
EPS = 1e-5
NCORES = 8
GPG = G // NCORES  # graphs per core = 4
SG = 6             # edge tiles per activation supergroup
SPL = 24           # edge tiles per gather split
_LEAD = 6          # lead split size for block 0
_DEFER = 3         # supergroup index to release deferred const loads

BF16 = ml_dtypes.bfloat16
F8 = ml_dtypes.float8_e4m3


def _wrap_idx(idx):
    """[n] int -> [16, n//16] int16 in the gather engine's wrapped layout
    (replicated to 128 partitions on device)."""
    n = idx.shape[0]
    assert n % 16 == 0
    return np.ascontiguousarray(idx.reshape(n // 16, 16).T.astype(np.int16))


def _splits(T, lead=0):
    out = []
    t0 = 0
    if lead and T > lead + 4:
        out.append((0, lead))
        t0 = lead
    rest = T - t0
    nsp = (rest + SPL - 1) // SPL
    base = rest // nsp
    rem = rest % nsp
    for i in range(nsp):
        seg = base + (1 if i < rem else 0)
        out.append((t0, seg))
        t0 += seg
    return out


def _preprocess(x, node_batch, edge_index, edge_attr):
    nb = np.asarray(node_batch)
    ei = np.asarray(edge_index)
    NN = nb.shape[0]
    counts = np.bincount(nb, minlength=G)
    gstart = np.concatenate([[0], np.cumsum(counts)[:-1]])
    GPAD = max(128, int(np.ceil(counts.max() / 128)) * 128)
    NPAD = GPG * GPAD
    NB = NPAD // 128
    PTOT = NCORES * NPAD
    assert PTOT < 32768

    g_of = nb  # nodes sorted by graph
    core_of_node = g_of // GPG
    slot = (g_of % GPG) * GPAD + (np.arange(NN) - gstart[g_of])
    pad_slot = core_of_node * NPAD + slot  # global padded index
    # gather-table rows use a (j, block)-interleaved order so one DMA ships a
    # whole graph's [128, NBG, C] SBUF tile of transposed blocks
    NBG = GPAD // 128
    _s = np.arange(NPAD)
    rp_of_slot = (_s // GPAD) * GPAD + (_s % 128) * NBG + (_s % GPAD) // 128
    pad_rp = core_of_node * NPAD + rp_of_slot[slot]

    src, dst = ei[0], ei[1]
    core_of_edge = core_of_node[dst]
    dst_local = pad_slot[dst] - core_of_edge * NPAD

    # per (core, block) edge lists
    order = np.lexsort((dst_local, core_of_edge))
    src_s, dst_s = src[order], dst_local[order]
    core_s = core_of_edge[order]
    blk_s = dst_s // 128
    cb = np.zeros((NCORES, NB), np.int64)
    np.add.at(cb, (core_s, blk_s), 1)
    T_b = [int(np.ceil(cb[:, b].max() / 128)) for b in range(NB)]
    TT = sum(T_b)
    EP = TT * 128

    core_edge_start = np.concatenate([[0], np.cumsum(np.bincount(core_s, minlength=NCORES))])
    per_core = []
    x_np = np.asarray(x)
    ea = np.asarray(edge_attr)

    for c in range(NCORES):
        lo, hi = core_edge_start[c], core_edge_start[c + 1]
        srcc, dstc, blkc = src_s[lo:hi], dst_s[lo:hi], blk_s[lo:hi]
        eidc = order[lo:hi]
        # slot edges into padded per-block tile space
        src_pad = np.zeros(EP, np.int64)
        dstrel = np.full(EP, -1, np.int64)
        attr_sel = np.zeros(EP, np.int64)
        attr_valid = np.zeros(EP, bool)
        off = 0
        boff = np.concatenate([[0], np.cumsum(np.bincount(blkc, minlength=NB))])
        for b in range(NB):
            cnt = boff[b + 1] - boff[b]
            sl = slice(boff[b], boff[b + 1])
            src_pad[off:off + cnt] = pad_rp[srcc[sl]]
            dstrel[off:off + cnt] = dstc[sl] - 128 * b
            attr_sel[off:off + cnt] = eidc[sl]
            attr_valid[off:off + cnt] = True
            off += T_b[b] * 128
        assert off == EP

        attrT = np.zeros((65, EP), BF16)
        attrT[:D, attr_valid] = ea[attr_sel[attr_valid]].astype(BF16).T
        attrT[64, :] = BF16(1.0)

        # one-hots (fp8, exact 0/1)
        epos = np.arange(EP)
        valid = dstrel >= 0
        oh_dst = np.zeros((128, EP), F8)
        oh_dst[dstrel[valid], epos[valid]] = F8(1.0)
        # oh_scat[p, t*128 + n] = 1 iff edge (tile t, slot p) has dstrel == n
        oh_scat = np.zeros((128, EP), F8)
        t_of = epos // 128
        p_of = epos % 128
        oh_scat[p_of[valid], t_of[valid] * 128 + dstrel[valid]] = F8(1.0)

        xcn = np.zeros((C, NPAD), np.float32)
        mask = np.zeros((1, NPAD), np.float32)
        own = core_of_node == c
        xcn[:, slot[own]] = x_np[own].T
        mask[0, slot[own]] = 1.0
        xncbf = np.zeros((NPAD, C), BF16)
        xncbf[rp_of_slot[slot[own]]] = x_np[own].astype(BF16)
        invcnt = (1.0 / (np.maximum(counts[c * GPG:(c + 1) * GPG], 1) * C)).astype(np.float32)

        per_core.append(dict(
            srcidx=np.ascontiguousarray(np.tile(_wrap_idx(src_pad), (8, 1))),
            attrt=attrT,
            ohdst=oh_dst,
            ohsct=oh_scat,
            xcn=xcn,
            xbfc=xcn.astype(BF16),
            mask=np.ascontiguousarray(np.broadcast_to(mask.astype(BF16), (128, NPAD))),
            invcntb=np.ascontiguousarray(np.broadcast_to(invcnt.reshape(1, GPG), (128, GPG))),
            xncbf=xncbf,
        ))

    meta = dict(GPAD=GPAD, NPAD=NPAD, NB=NB, PTOT=PTOT, T_b=T_b, TT=TT, EP=EP,
                pad_slot=pad_slot, counts=counts, NN=NN, rp_of_slot=rp_of_slot)
    return per_core, meta


def _prep_weights(Wf, bf, Ws, bs, W1, b1, g1, be1, W2, b2, lnw, lnb):
    # z-part weight blocks: columns 0:128 gate (negated for exp(-g)), 128:256 core
    wdst = np.zeros((L, 128, 256), np.float32)
    wsrc = np.zeros((L, 128, 256), np.float32)
    wa = np.zeros((L, 65, 256), np.float32)
    for l in range(L):
        wdst[l, :, 0:128] = -Wf[l][:, 0:C].T
        wdst[l, :, 128:256] = Ws[l][:, 0:C].T
        wsrc[l, :, 0:128] = -Wf[l][:, C:2 * C].T
        wsrc[l, :, 128:256] = Ws[l][:, C:2 * C].T
        wa[l, :D, 0:128] = -Wf[l][:, 2 * C:].T
        wa[l, :D, 128:256] = Ws[l][:, 2 * C:].T
        wa[l, 64, 0:128] = -bf[l]
        wa[l, 64, 128:256] = bs[l]
    w1t = np.stack([np.stack([W1[l].T[:, 128 * k:128 * (k + 1)] for k in range(4)]) for l in range(L)])
    w2t = np.stack([np.stack([W2[l].T[128 * k:128 * (k + 1), :] for k in range(4)]) for l in range(L)])
    b1c = np.asarray(b1, np.float32).reshape(L, 4, 128).transpose(2, 0, 1).reshape(128, L * 4)
    g1c = np.asarray(g1, np.float32).reshape(L, 4, 128).transpose(2, 0, 1).reshape(128, L * 4)
    be1c = np.asarray(be1, np.float32).reshape(L, 4, 128).transpose(2, 0, 1).reshape(128, L * 4)
    b2c = np.asarray(b2, np.float32).T
    lnwc = np.asarray(lnw, np.float32).T
    lnbc = np.asarray(lnb, np.float32).T
    consts = np.ascontiguousarray(np.concatenate(
        [b1c, g1c, be1c, b2c, lnwc, lnbc], axis=1))  # [128, 3*4L + 3*L]
    return dict(
        wdst=np.ascontiguousarray(wdst.transpose(1, 0, 2).reshape(128, L * 256)).astype(BF16),
        wsrc=np.ascontiguousarray(wsrc.transpose(1, 0, 2).reshape(128, L * 256)).astype(BF16),
        wa=np.ascontiguousarray(wa.transpose(1, 0, 2).reshape(65, L * 256)).astype(BF16),
        w1t=np.ascontiguousarray(w1t.reshape(L * 4, 128, 128).transpose(1, 0, 2).reshape(128, L * 4 * 128)).astype(BF16),
        w2t=np.ascontiguousarray(w2t.reshape(L * 4, 128, 128).transpose(1, 0, 2).reshape(128, L * 4 * 128)).astype(BF16),
        consts=consts,
    )


def _trace(meta, nlayers=L, use_cc=True, debug_stage=None):
    from concourse import bacc, mybir, bass_isa
    import concourse.tile as tile

    # Force every activation onto the exp+ln table (index 6) so the ACT
    # table-load pass never alternates tables between Exp and Ln ops.
    if not hasattr(bacc, "_orig_get_act_tables"):
        bacc._orig_get_act_tables = bacc.get_activation_tables

        def _only_table6(arch):
            tabs = bacc._orig_get_act_tables(arch)
            out = {}
            for name, funcs in tabs.items():
                out[name] = funcs if name == "natural_log_exp_and_others" else set()
            return out

        bacc.get_activation_tables = _only_table6

    F32 = mybir.dt.float32
    B16 = mybir.dt.bfloat16
    F8E4 = mybir.dt.float8e4
    I16 = mybir.dt.int16
    AF = mybir.ActivationFunctionType
    OP = mybir.AluOpType

    NPAD, NB, PTOT, TT, EP = meta["NPAD"], meta["NB"], meta["PTOT"], meta["TT"], meta["EP"]
    GPAD = meta["GPAD"]
    T_b = meta["T_b"]
    NN = meta["NN"]

    nc = bacc.Bacc("TRN2", target_bir_lowering=False, debug=False, num_devices=NCORES)

    def din(name, shape, dt):
        return nc.dram_tensor(name, shape, dt, kind="ExternalInput").ap()

    xcn_in = din("xcn", [C, NPAD], F32)
    xbfc_in = din("xbfc", [C, NPAD], B16)
    xncbf_in = din("xncbf", [NPAD, C], B16)
    srcidx_in = din("srcidx", [128, TT * 8], I16)
    attrt_in = din("attrt", [65, EP], B16)
    ohdst_in = din("ohdst", [128, EP], F8E4)
    ohsct_in = din("ohsct", [128, EP], F8E4)
    mask_in = din("mask", [128, NPAD], B16)
    invcntb_in = din("invcntb", [128, GPG], F32)
    ident_in = din("ident", [128, 128], F32)
    identb_in = din("identb", [128, 128], B16)
    wdst_in = din("wdst", [128, L * 256], B16)
    wsrc_in = din("wsrc", [128, L * 256], B16)
    wa_in = din("wa", [65, L * 256], B16)
    w1t_in = din("w1t", [128, L * 4 * 128], B16)
    w2t_in = din("w2t", [128, L * 4 * 128], B16)
    consts_in = din("consts", [128, 15 * L], F32)
    xout = nc.dram_tensor("xout", [NPAD, C], F32, kind="ExternalOutput").ap()

    splits = _splits

    with tile.TileContext(nc) as tc:
        with (
            tc.tile_pool(name="const", bufs=1) as cp,
            tc.tile_pool(name="xstate", bufs=2) as xp,
            tc.tile_pool(name="dram", bufs=1, space="DRAM") as dr,
        ):
            # ---- constants (critical-path loads first, chunked across
            # queues so no single 60us serial DMA gates the edge phase) ----
            NCH = 8
            if use_cc:
                xtab_start = dr.tile([PTOT, C], B16, tag="xtab0", addr_space="Shared")
            else:
                xtab_start = dr.tile([PTOT, C], B16, tag="xtab0")
            if use_cc:
                tab0_b = dr.tile([NPAD, C], B16, tag="tab0b")
                for i in range(NCH):
                    r0, r1 = NPAD * i // NCH, NPAD * (i + 1) // NCH
                    eng = nc.sync if i % 2 == 0 else nc.scalar
                    eng.dma_start(out=tab0_b[r0:r1, :], in_=xncbf_in[r0:r1, :])
                nc.gpsimd.collective_compute(
                    "AllGather", OP.bypass, replica_groups=[list(range(NCORES))],
                    ins=[tab0_b[:].opt()], outs=[xtab_start[:].opt()])
            else:
                for i in range(NCH):
                    r0, r1 = NPAD * i // NCH, NPAD * (i + 1) // NCH
                    eng = nc.sync if i % 2 == 0 else nc.scalar
                    eng.dma_start(out=xtab_start[r0:r1, :], in_=xncbf_in[r0:r1, :])
            srcidx = cp.tile([128, TT * 8], I16)
            for i in range(4):
                c0, c1 = TT * 8 * i // 4, TT * 8 * (i + 1) // 4
                nc.gpsimd.dma_start(out=srcidx[:, c0:c1], in_=srcidx_in[:, c0:c1])
            x_bf = xp.tile([C, NPAD], B16, tag="xbf", bufs=1, name="xbf")
            for i in range(4):
                c0, c1 = NPAD * i // 4, NPAD * (i + 1) // 4
                nc.gpsimd.dma_start(out=x_bf[:, c0:c1], in_=xbfc_in[:, c0:c1])
            wdst = cp.tile([128, L, 256], B16)
            wsrc = cp.tile([128, L, 256], B16)
            wa = cp.tile([65, L, 256], B16)
            nc.scalar.dma_start(out=wdst[:], in_=wdst_in[:])
            nc.scalar.dma_start(out=wsrc[:], in_=wsrc_in[:])
            nc.scalar.dma_start(out=wa[:], in_=wa_in[:])

            # loads not needed until mid-edge-phase are deferred: their
            # dma_starts are emitted a few supergroups into layer 0 so the
            # DMA engines prioritise the first gather/one-hot transfers
            x_fp = xp.tile([C, NPAD], F32, tag="xfp", bufs=1, name="xfp")
            U_all = xp.tile([128, NB, 2, 256], F8E4, tag="uall", bufs=1, name="uall")
            mask = cp.tile([128, NPAD], B16)
            invcntb = cp.tile([128, GPG], F32)
            ident = cp.tile([128, 128], F32)
            identb = cp.tile([128, 128], B16)
            w1t = cp.tile([128, L * 4, 128], B16)
            w2t = cp.tile([128, L * 4, 128], B16)
            allc = cp.tile([128, 15 * L], F32)

            def _deferred_loads():
                for i in range(4):
                    c0, c1 = NPAD * i // 4, NPAD * (i + 1) // 4
                    nc.scalar.dma_start(out=x_fp[:, c0:c1], in_=xcn_in[:, c0:c1])
                nc.scalar.dma_start(out=mask[:], in_=mask_in[:])
                nc.scalar.dma_start(out=invcntb[:], in_=invcntb_in[:])
                nc.scalar.dma_start(out=ident[:], in_=ident_in[:])
                nc.scalar.dma_start(out=identb[:], in_=identb_in[:])
                nc.scalar.dma_start(out=w1t[:], in_=w1t_in[:])
                nc.scalar.dma_start(out=w2t[:], in_=w2t_in[:])
                nc.scalar.dma_start(out=allc[:], in_=consts_in[:])

            deferred = [_deferred_loads]
            b1c = allc[:, 0:4 * L]
            g1c = allc[:, 4 * L:8 * L]
            be1c = allc[:, 8 * L:12 * L]
            b2c = allc[:, 12 * L:13 * L]
            lnwc = allc[:, 13 * L:14 * L]
            lnbc = allc[:, 14 * L:15 * L]
            epsc = cp.tile([128, 1], F32)
            nc.gpsimd.memset(epsc[:], EPS)
            agouts = []
            agins = []
            for l in range(nlayers):
                tab = xtab_start[:] if l == 0 else agouts[l - 1][:]
                if l == 0:
                    # dst-side preacts per block (later layers fold this into
                    # the previous node phase, per graph)
                    with tc.tile_pool(name="upsum", bufs=2, space="PSUM") as up:
                        for b in range(NB):
                            ups = up.tile([128, 256], F32, tag="ups", space="PSUM")
                            nc.tensor.matmul(out=ups[:], lhsT=x_bf[:, 128 * b:128 * (b + 1)],
                                             rhs=wdst[:, l, :], start=True, stop=True)
                            nc.vector.tensor_scalar(out=U_all[:, b, 0, :], in0=ups[:],
                                                    scalar1=1.0, scalar2=0.0, op0=OP.mult, op1=OP.add)
                            nc.vector.tensor_tensor(out=U_all[:, b, 1, :], in0=ups[:],
                                                    in1=U_all[:, b, 0, :], op=OP.subtract)
                # ---------------- edge phase ----------------
                # h chunks for the BN pass, interleaved into the edge phase
                # (256-wide so two chunks ping-pong within one PSUM bank)
                NBG = GPAD // 128
                NCHK = (GPAD + 511) // 512
                hchunks = []
                cpb = GPAD // 128 // NCHK
                mlo = 0
                for ci in range(NCHK):
                    wblk = cpb + (1 if ci < (GPAD // 128) % NCHK else 0)
                    hchunks.append((mlo, wblk * 128))
                    mlo += wblk * 128
                x2_fp = xp.tile([C, NPAD], F32, tag="x2fp", bufs=1, name=f"x2fp_{l}")
                x2_bf = xp.tile([C, NPAD], B16, tag="x2bf", bufs=1, name=f"x2bf_{l}")
                h_bf = xp.tile([128, 4, NPAD], B16, tag="hbf", bufs=1, name=f"hbf_{l}")
                s12p = xp.tile([128, 2, 4, NCHK * GPG], F32, tag="s12p", bufs=1,
                               name=f"s12p_{l}")
                with (
                    tc.tile_pool(name="egath", bufs=3) as gp,
                    tc.tile_pool(name="eact", bufs=3) as ep,
                    tc.tile_pool(name="epsum", bufs=2, space="PSUM") as pp,
                    tc.tile_pool(name="aggpsum", bufs=1, space="PSUM") as ap_,
                    tc.tile_pool(name="hwork", bufs=3) as hw,
                ):
                    # flat supergroup schedule: for each entry we emit
                    # preacts(i) on PE BEFORE scatter(i-1), so the PE never
                    # stalls waiting for the ACT/DVE message chain
                    sched = []
                    toff = 0
                    for b in range(NB):
                        T = T_b[b]
                        if T == 0:
                            sched.append(dict(b=b, empty=True))
                            toff += T
                            continue
                        for (ts0, seg) in splits(T, lead=_LEAD if b == 0 else 0):
                            t0 = ts0
                            first_split = ts0 == 0
                            while t0 < ts0 + seg:
                                sgn = min(SG, ts0 + seg - t0)
                                sched.append(dict(
                                    b=b, empty=False, T=T, toff=toff,
                                    ts0=ts0, seg=seg, t0=t0, sgn=sgn,
                                    new_split=(t0 == ts0),
                                    first=(first_split and t0 == ts0),
                                    last=(t0 + sgn == T)))
                                t0 += sgn
                        toff += T

                    cur_split = {}
                    pend = None  # (sg_entry, msg_tile, split_tiles)
                    aggpair = ap_.tile([128, 2, 128], F32, tag="aggpair", space="PSUM",
                                       name=f"aggpair_{l}")
                    hpp = ap_.tile([128, 1, 512], F32, tag="hpsum", space="PSUM",
                                   name=f"hpsum_{l}")
                    hqueue = []   # pending h-chunk thunks
                    blocks_done = [0] * NB
                    hslot = [0]

                    def emit_hchunk(g, k, ci, drain):
                        mlo, w = hchunks[ci]
                        glo = g * GPAD
                        sl_ = 0
                        nc.tensor.matmul(out=hpp[:, sl_, 0:w], lhsT=w1t[:, 4 * l + k, :],
                                         rhs=x2_bf[:, glo + mlo:glo + mlo + w],
                                         start=True, stop=True)
                        sco = slice(g * NCHK + ci, g * NCHK + ci + 1)
                        nc.vector.tensor_scalar(out=h_bf[:, k, glo + mlo:glo + mlo + w],
                                                in0=hpp[:, sl_, 0:w], scalar1=1.0, scalar2=0.0,
                                                op0=OP.mult, op1=OP.add,
                                                accum_out=s12p[:, 0, k, sco])
                        junk = hw.tile([128, 512], B16, tag="junk")
                        hsl = h_bf[:, k, glo + mlo:glo + mlo + w]
                        if drain:
                            nc.scalar.activation(junk[:, 0:w], hsl, AF.Square,
                                                 accum_out=s12p[:, 1, k, sco])
                        else:
                            nc.vector.scalar_tensor_tensor(out=junk[:, 0:w], in0=hsl,
                                                           scalar=0.0, in1=hsl,
                                                           op0=OP.add, op1=OP.mult,
                                                           accum_out=s12p[:, 1, k, sco])

                    def on_block_done(bp):
                        # queue h chunks whose column span is now complete
                        blocks_done[bp] = 1
                        g = bp // NBG
                        bl = bp - g * NBG
                        for ci, (mlo, w) in enumerate(hchunks):
                            if (mlo + w - 1) // 128 == bl:
                                for k in range(4):
                                    hqueue.append((g, k, ci))

                    def emit_scatter(p):
                        sgp, msgp, spl = p
                        bp = sgp["b"]
                        agg = aggpair[:, bp % 2, :]
                        for t in range(sgp["t0"], sgp["t0"] + sgp["sgn"]):
                            s = t - sgp["t0"]
                            nsl = slice((t - sgp["ts0"]) * 128, (t - sgp["ts0"] + 1) * 128)
                            nc.tensor.matmul(out=agg, lhsT=msgp[:, s, :], rhs=spl["ohs"][:, nsl],
                                             start=(t == 0), stop=(t == sgp["T"] - 1))
                        if sgp["last"]:
                            bcol = slice(128 * bp, 128 * (bp + 1))
                            nc.vector.tensor_tensor(out=x2_fp[:, bcol], in0=x_fp[:, bcol],
                                                    in1=agg, op=OP.add)
                            nc.vector.tensor_scalar(out=x2_bf[:, bcol], in0=x2_fp[:, bcol],
                                                    scalar1=1.0, scalar2=0.0, op0=OP.mult, op1=OP.add)
                            on_block_done(bp)

                    for sgi, sg in enumerate(sched):
                        if sgi == _DEFER and deferred:
                            deferred.pop()()
                        b = sg["b"]
                        if sg["empty"]:
                            bcol = slice(128 * b, 128 * (b + 1))
                            nc.vector.tensor_copy(x2_fp[:, bcol], x_fp[:, bcol])
                            nc.vector.tensor_copy(x2_bf[:, bcol], x_fp[:, bcol])
                            on_block_done(b)
                            continue
                        if sg["new_split"]:
                            ts0, seg, toff = sg["ts0"], sg["seg"], sg["toff"]
                            ni = seg * 128
                            e0 = (toff + ts0) * 128
                            zsrc = gp.tile([128, 1, ni], B16, tag="zsrc")
                            nc.gpsimd.dma_gather(zsrc[:], tab, srcidx[:, (toff + ts0) * 8:(toff + ts0 + seg) * 8],
                                                 num_idxs=ni, num_idxs_reg=ni, elem_size=C,
                                                 transpose=True, single_packet=False)
                            attr = gp.tile([65, ni], B16, tag="attr")
                            nc.sync.dma_start(out=attr[:], in_=attrt_in[:, e0:e0 + ni])
                            ohd = gp.tile([128, 1, ni], F8E4, tag="ohd")
                            nc.sync.dma_start(out=ohd[:], in_=ohdst_in[:, e0:e0 + ni])
                            ohs = gp.tile([128, ni], F8E4, tag="ohs")
                            nc.sync.dma_start(out=ohs[:], in_=ohsct_in[:, e0:e0 + ni])
                            cur_split = dict(zsrc=zsrc, attr=attr, ohd=ohd, ohs=ohs)
                        spl = cur_split
                        sgn, t0, ts0 = sg["sgn"], sg["t0"], sg["ts0"]
                        pre = pp.tile([128, SG, 256], F32, tag="pre", space="PSUM")
                        for t in range(t0, t0 + sgn):
                            s = t - t0
                            esl = slice((t - ts0) * 128, (t - ts0 + 1) * 128)
                            nc.tensor.matmul(out=pre[:, s, :],
                                             lhsT=spl["ohd"][:, :, esl].broadcast_to([128, 2, 128]),
                                             rhs=U_all[:, b, :, :], start=True, stop=False,
                                             perf_mode=mybir.MatmulPerfMode.DoubleRow)
                            nc.tensor.matmul(out=pre[:, s, :], lhsT=spl["zsrc"][:, 0, esl],
                                             rhs=wsrc[:, l, :], start=False, stop=False)
                            nc.tensor.matmul(out=pre[:, s, :], lhsT=spl["attr"][0:65, esl],
                                             rhs=wa[0:65, l, :], start=False, stop=True)
                        if pend is not None:
                            emit_scatter(pend)
                            if hqueue:
                                g_, k_, ci_ = hqueue.pop(0)
                                emit_hchunk(g_, k_, ci_, drain=False)
                        uv = ep.tile([128, SG, 256], B16, tag="uv", bufs=4)
                        sp = ep.tile([128, SG, 128], B16, tag="sp")
                        t32 = ep.tile([128, SG, 128], F32, tag="t32", bufs=4)
                        r32 = ep.tile([128, SG, 128], F32, tag="r32", bufs=4)
                        msg = ep.tile([128, SG, 128], B16, tag="msg")
                        nc.scalar.activation(uv[:, :sgn, :], pre[:, :sgn, :], AF.Exp)
                        nc.scalar.activation(sp[:, :sgn, :], uv[:, :sgn, C:2 * C], AF.Ln, bias=1.0)
                        if sg["new_split"]:
                            nc.vector.tensor_scalar(out=t32[:, :sgn, :], in0=uv[:, :sgn, 0:C],
                                                    scalar1=1.0, scalar2=None, op0=OP.add)
                        else:
                            nc.gpsimd.tensor_scalar_add(t32[:, :sgn, :], uv[:, :sgn, 0:C], 1.0)
                        nc.vector.reciprocal_approx_fast(out=r32[:, :sgn, :], in_=t32[:, :sgn, :])
                        nc.vector.tensor_tensor(out=msg[:, :sgn, :], in0=sp[:, :sgn, :],
                                                in1=r32[:, :sgn, :], op=OP.mult)
                        pend = (sg, msg, spl)
                    if pend is not None:
                        emit_scatter(pend)
                        pend = None
                    while hqueue:
                        g_, k_, ci_ = hqueue.pop(0)
                        emit_hchunk(g_, k_, ci_, drain=True)

                if debug_stage == "x2":
                    with tc.tile_pool(name="dbg", bufs=2, space="PSUM") as dbp:
                        with tc.tile_pool(name="dbw", bufs=2) as dbw:
                            for b in range(NB):
                                tpd = dbp.tile([128, 128], F32, tag="dtp", space="PSUM")
                                nc.tensor.transpose(out=tpd[:], in_=x2_fp[:, 128 * b:128 * (b + 1)],
                                                    identity=ident[:])
                                xo = dbw.tile([128, 128], F32, tag="dxo")
                                nc.vector.tensor_copy(xo[:], tpd[:])
                                nc.sync.dma_start(out=xout[128 * b:128 * (b + 1), :], in_=xo[:])
                    break
                # ---------------- node phase ----------------
                with tc.tile_pool(name="nsb", bufs=1) as np_:
                    bnstat = np_.tile([128, 8], F32)
                    nc.vector.tensor_reduce(out=bnstat[:], in_=s12p[:], axis=mybir.AxisListType.X, op=OP.add)
                    bns = np_.tile([128, 8], F32)
                    if use_cc:
                        bnin = dr.tile([128, 8], F32, tag="bnin", bufs=2)
                        bnout = dr.tile([128, 8], F32, tag="bnout", bufs=2, addr_space="Shared")
                        nc.sync.dma_start(out=bnin[:], in_=bnstat[:])
                        nc.gpsimd.collective_compute(
                            "AllReduce", OP.add, replica_groups=[list(range(NCORES))],
                            ins=[bnin[:].opt()], outs=[bnout[:].opt()])
                        nc.sync.dma_start(out=bns[:], in_=bnout[:])
                    else:
                        nc.vector.tensor_scalar(out=bns[:], in0=bnstat[:], scalar1=float(NCORES),
                                                scalar2=None, op0=OP.mult)
                    ksl = slice(4 * l, 4 * l + 4)
                    mean_r = np_.tile([128, 4], F32)
                    nc.vector.tensor_scalar(out=mean_r[:], in0=bns[:, 0:4], scalar1=1.0 / NN,
                                            scalar2=None, op0=OP.mult)
                    var = np_.tile([128, 4], F32)
                    nc.vector.tensor_scalar(out=var[:], in0=bns[:, 4:8], scalar1=1.0 / NN,
                                            scalar2=None, op0=OP.mult)
                    msq = np_.tile([128, 4], F32)
                    nc.vector.tensor_tensor(out=msq[:], in0=mean_r[:], in1=mean_r[:], op=OP.mult)
                    nc.vector.tensor_tensor(out=var[:], in0=var[:], in1=msq[:], op=OP.subtract)
                    rstd = np_.tile([128, 4], F32)
                    nc.scalar.activation(rstd[:], var[:], AF.Ln, bias=epsc[:])
                    nc.scalar.activation(rstd[:], rstd[:], AF.Exp, scale=-0.5)
                    a_bn = np_.tile([128, 4], F32)
                    nc.vector.tensor_tensor(out=a_bn[:], in0=rstd[:], in1=g1c[:, ksl], op=OP.mult)
                    inva = np_.tile([128, 4], F32)
                    nc.vector.reciprocal_approx_fast(out=inva[:], in_=a_bn[:])
                    # q = be1/a + b1 - (mean_r + b1) = be1/a - mean_r
                    q = np_.tile([128, 4], F32)
                    nc.vector.tensor_tensor(out=q[:], in0=be1c[:, ksl], in1=inva[:], op=OP.mult)
                    nc.vector.tensor_tensor(out=q[:], in0=q[:], in1=mean_r[:], op=OP.subtract)
                    w2s = np_.tile([128, 4, 128], B16)
                    for k in range(4):
                        nc.vector.tensor_scalar(out=w2s[:, k, :], in0=w2t[:, 4 * l + k, :],
                                                scalar1=a_bn[:, k:k + 1], scalar2=0.0,
                                                op0=OP.mult, op1=OP.add)

                    if l < nlayers - 1:
                        if use_cc:
                            agin = dr.tile([NPAD, C], B16, tag="agin", bufs=2)
                            agout = dr.tile([PTOT, C], B16, tag="agout", bufs=2,
                                            addr_space="Shared")
                            tabw = agin
                        else:
                            # metric/sim mode: write the local table rows
                            # directly, no gather copy
                            agout = dr.tile([PTOT, C], B16, tag="agout", bufs=2)
                            agin = None
                            tabw = agout
                        agins.append(agin)
                    y_fp = xp.tile([C, NPAD], F32, tag="xfp", bufs=1, name=f"yfp_{l}")
                    if l < nlayers - 1:
                        y_bf = xp.tile([C, NPAD], B16, tag="xbf", bufs=1, name=f"ybf_{l}")
                    with (
                        tc.tile_pool(name="xwork", bufs=6) as nw,
                        tc.tile_pool(name="xpsum", bufs=2, space="PSUM") as xpp_,
                        tc.tile_pool(name="tpsum", bufs=2, space="PSUM") as tp_,
                    ):
                        # pass A: per graph, hn + W2 matmuls + x3m + LN sums
                        gx3m = []
                        mvall = np_.tile([128, 2, GPG], F32, tag="mvall", bufs=1,
                                         name=f"mvall_{l}")
                        m2g = np_.tile([128, GPG], F32, tag="m2g", bufs=2)
                        vgg = np_.tile([128, GPG], F32, tag="vgg", bufs=2)
                        rgg = np_.tile([128, GPG], F32, tag="rgg", bufs=2)
                        scalb = np_.tile([128, GPG], F32, tag="scalb", bufs=2)
                        biasb = np_.tile([128, GPG], F32, tag="biasb", bufs=2)
                        # pass B+C: per graph, y -> ship
                        def emit_ship(g):
                            glo = g * GPAD
                            gsl = slice(glo, glo + GPAD)
                            x3m = gx3m[g]
                            # y = (x3m * scal + bias) * mask  (pads stay exactly 0)
                            jacc = np_.tile([128, 1], F32, tag="jacc", bufs=4,
                                            name=f"jacc_{l}_{g}")
                            nc.vector.affine_mul_reduce(out=y_fp[:, gsl], accum_out=jacc[:],
                                                        in0=x3m[:], in1=mask[:, gsl],
                                                        scale=scalb[:, g:g + 1], bias=biasb[:, g:g + 1])
                            if l < nlayers - 1:
                                nc.gpsimd.tensor_copy(y_bf[:, gsl], y_fp[:, gsl])
                                # ship this graph's blocks to the gather table:
                                # table rows are (j, block)-interleaved so all
                                # NBG transposed blocks go out as ONE DMA
                                xnc = nw.tile([128, NBG, 128], B16, tag="xnc")
                                for bb in range(NBG):
                                    gb = glo + 128 * bb
                                    tp = tp_.tile([128, 128], B16, tag="tp", space="PSUM", bufs=2)
                                    nc.tensor.transpose(out=tp[:], in_=y_bf[:, gb:gb + 128],
                                                        identity=identb[:])
                                    if bb % 2 == 0:
                                        nc.scalar.activation(xnc[:, bb, :], tp[:], AF.Copy)
                                    else:
                                        nc.vector.tensor_scalar(out=xnc[:, bb, :], in0=tp[:], scalar1=1.0,
                                                                scalar2=0.0, op0=OP.mult, op1=OP.add)
                                nc.sync.dma_start(out=tabw[glo:glo + GPAD, :], in_=xnc[:])
                                # next layer's dst-side preacts for this graph
                                for bb in range(NBG):
                                    gb = glo + 128 * bb
                                    b = gb // 128
                                    ups = tp_.tile([128, 256], F32, tag="ups", space="PSUM")
                                    nc.tensor.matmul(out=ups[:], lhsT=y_bf[:, gb:gb + 128],
                                                     rhs=wdst[:, l + 1, :], start=True, stop=True)
                                    if bb % 2 == 0:
                                        nc.vector.tensor_scalar(out=U_all[:, b, 0, :], in0=ups[:],
                                                                scalar1=1.0, scalar2=0.0, op0=OP.mult, op1=OP.add)
                                    else:
                                        nc.scalar.activation(U_all[:, b, 0, :], ups[:], AF.Copy)
                                    nc.vector.tensor_tensor(out=U_all[:, b, 1, :], in0=ups[:],
                                                            in1=U_all[:, b, 0, :], op=OP.subtract)
                            else:
                                xnc32 = nw.tile([128, NBG, 128], F32, tag="xnc32")
                                for bb in range(NBG):
                                    gb = glo + 128 * bb
                                    tp = tp_.tile([128, 128], F32, tag="tpf", space="PSUM", bufs=2)
                                    nc.tensor.transpose(out=tp[:], in_=y_fp[:, gb:gb + 128],
                                                        identity=ident[:])
                                    if bb % 2 == 0:
                                        nc.scalar.activation(xnc32[:, bb, :], tp[:], AF.Copy)
                                    else:
                                        nc.vector.tensor_scalar(out=xnc32[:, bb, :], in0=tp[:], scalar1=1.0,
                                                                scalar2=0.0, op0=OP.mult, op1=OP.add)
                                nc.sync.dma_start(out=xout[glo:glo + GPAD, :], in_=xnc32[:])

                        ship_q = []
                        for g in range(GPG):
                            glo = g * GPAD
                            gsl = slice(glo, glo + GPAD)
                            xpp = xpp_.tile([128, GPAD], F32, tag="xpp", space="PSUM")
                            for k in range(4):
                                hn = nw.tile([128, GPAD], B16, tag="hn")
                                if k % 2 == 0:
                                    nc.vector.tensor_scalar(out=hn[:], in0=h_bf[:, k, gsl],
                                                            scalar1=q[:, k:k + 1], scalar2=0.0,
                                                            op0=OP.add, op1=OP.max)
                                else:
                                    nc.gpsimd.tensor_scalar(out=hn[:], in0=h_bf[:, k, gsl],
                                                            scalar1=q[:, k:k + 1], scalar2=0.0,
                                                            op0=OP.add, op1=OP.max)
                                for mlo in range(0, GPAD, 512):
                                    w = min(512, GPAD - mlo)
                                    nc.tensor.matmul(out=xpp[:, mlo:mlo + w], lhsT=w2s[:, k, :],
                                                     rhs=hn[:, mlo:mlo + w], start=(k == 0), stop=False)
                            for mlo in range(0, GPAD, 512):
                                w = min(512, GPAD - mlo)
                                nc.tensor.matmul(out=xpp[:, mlo:mlo + w], lhsT=identb[:],
                                                 rhs=x2_bf[:, glo + mlo:glo + mlo + w],
                                                 start=False, stop=True)
                            # x3 = (xpp + b2) * mask, with row-sums for LN
                            x3m = nw.tile([128, GPAD], F32, tag="x3m", bufs=6)
                            ls = np_.tile([128, 2], F32, tag="ls", bufs=6)
                            nc.vector.affine_mul_reduce(out=x3m[:], accum_out=ls[:, 0:1],
                                                        in0=xpp[:], in1=mask[:, gsl],
                                                        scale=1.0, bias=b2c[:, l:l + 1])
                            junk2 = nw.tile([128, GPAD], B16, tag="junk2")
                            nc.scalar.activation(junk2[:], x3m[:], AF.Square,
                                                 accum_out=ls[:, 1:2])
                            lsr = np_.tile([128, 2], F32, tag="lsr", bufs=6)
                            nc.gpsimd.partition_all_reduce(lsr[:], ls[:], channels=128,
                                                           reduce_op=bass_isa.ReduceOp.add)
                            nc.vector.tensor_scalar(out=mvall[:, :, g], in0=lsr[:],
                                                    scalar1=invcntb[:, g:g + 1],
                                                    scalar2=None, op0=OP.mult)
                            gx3m.append(x3m)
                            if g % 2 == 1:
                                # batched LN scalars per graph pair, so the
                                # first pair's y can ship while pass A runs
                                # on the later graphs
                                hs = slice(g - 1, g + 1)
                                nc.vector.tensor_tensor(out=m2g[:, hs], in0=mvall[:, 0, hs],
                                                        in1=mvall[:, 0, hs], op=OP.mult)
                                nc.vector.tensor_tensor(out=vgg[:, hs], in0=mvall[:, 1, hs],
                                                        in1=m2g[:, hs], op=OP.subtract)
                                nc.scalar.activation(rgg[:, hs], vgg[:, hs], AF.Ln, bias=epsc[:])
                                nc.scalar.activation(rgg[:, hs], rgg[:, hs], AF.Exp, scale=-0.5)
                                nc.vector.tensor_scalar(out=scalb[:, hs], in0=rgg[:, hs],
                                                        scalar1=lnwc[:, l:l + 1],
                                                        scalar2=None, op0=OP.mult)
                                nc.vector.tensor_tensor(out=biasb[:, hs], in0=mvall[:, 0, hs],
                                                        in1=scalb[:, hs], op=OP.mult)
                                nc.vector.tensor_scalar(out=biasb[:, hs], in0=biasb[:, hs],
                                                        scalar1=-1.0,
                                                        scalar2=lnbc[:, l:l + 1],
                                                        op0=OP.mult, op1=OP.add)
                                ship_q.append(g - 1)
                                ship_q.append(g)
                                if g == GPG - 1:
                                    for gq in ship_q:
                                        emit_ship(gq)
                    if l < nlayers - 1:
                        agouts.append(agout)
                        if use_cc:
                            nc.gpsimd.collective_compute(
                                "AllGather", OP.bypass, replica_groups=[list(range(NCORES))],
                                ins=[agin[:].opt()], outs=[agout[:].opt()])
                x_fp = y_fp
                if l < nlayers - 1:
                    x_bf = y_bf

    nc.finalize()
    return nc


_CACHE = {}


def kernel(x, node_batch, edge_index, edge_attr,
           Wf, bf, Ws, bs, W1, b1, g1, be1, W2, b2, lnw, lnb):
    from concourse.bass_utils import run_bass_kernel_spmd

    per_core, meta = _preprocess(x, node_batch, edge_index, edge_attr)
    wd = _prep_weights(Wf, bf, Ws, bs, W1, b1, g1, be1, W2, b2, lnw, lnb)
    key = (meta["NPAD"], meta["NN"], tuple(meta["T_b"]))
    if key not in _CACHE:
        _CACHE[key] = _trace(meta)
    nc = _CACHE[key]

    ident = np.eye(128, dtype=np.float32)
    identb = np.eye(128, dtype=np.float32).astype(BF16)
    in_maps = []
    for c in range(NCORES):
        m = dict(per_core[c])
        m.update(wd)
        m.update(ident=ident, identb=identb)
        in_maps.append(m)
    res = run_bass_kernel_spmd(nc, in_maps, list(range(NCORES)))

    pad_slot = meta["pad_slot"]
    rp_of_slot = meta["rp_of_slot"]
    NPAD = meta["NPAD"]
    out = np.zeros((meta["NN"], C), np.float32)
    for c in range(NCORES):
        own = (pad_slot >= c * NPAD) & (pad_slot < (c + 1) * NPAD)
        out[own] = res.results[c]["xout"][rp_of_slot[pad_slot[own] - c * NPAD]]
    return out

